# revision 38
# baseline (speedup 1.0000x reference)
"""Edge-parallel GNN u_mul_v kernel for Trainium2 (8 NeuronCores).

z[e, :] = h[src[e], :] * h[dst[e], :]

Sharding: edges are globally sorted by src and cores take contiguous 100K
spans, so each core's src values live in a narrow ~6.3K-node window, shipped
per-core as rebased tables (indices fit the gather's signed int16).

Tables are dense fp16: two nodes per 256B row (h16 as [N_PAD/2, 128]). The
SWDGE gather cost is per descriptor with a 2x penalty under 512B, so a
128B descriptor (one node, elem_size=64 fp16, 256B row stride) costs half a
256B one, and a 512B descriptor (an aligned quad of 4 nodes) costs the same
as two 128B ones while serving up to 4 edge slots. The bass dma_gather
wrapper refuses sub-256B elements (a transpose-path restriction only), so
_gather128/_gather512 build InstDMAGatherAnt directly (the non-transpose
TX/RX descriptors in q7_kernels/extended_inst/dma_gather.cpp carry
arbitrary lengths; only the row stride is 256B-granular).

Per core the edges are split into:
  - quad region: greedy sets over dst quads (nodes 4q..4q+3) with >= 2
    participating nodes; one 512B dst descriptor serves the set's slots
    (descriptor m -> partition m%128, z levels 4*(m//128)+r). Descriptors
    are quad-ordered, so the dst stream stays sequential. The src side of
    these tiles gathers 128B rows from a padded per-core window table.
  - singles region: remaining edges, bucketed by (src parity, dst parity)
    and dst-sorted; both sides gather 128B rows via the even/odd column
    views of the packed tables.
Device programs are SPMD, so region/bucket capacities are padded to the
per-core max.

Per 8192-slot tile: two SWDGE gathers (separate queues), one DVE fp16
multiply, one contiguous HWDGE store of z [E_DEV, 64] fp16 (host converts
to fp32 and applies the inverse edge permutation). Gather index tensors are
shipped once (not 8x): queue q's Q7 pair reads idx only from partitions
[32q, 32q+32).
"""

import numpy as np

N_NODES = 50000
N_EDGES = 800000
D = 64
N_CORES = 8
E_PER_CORE = N_EDGES // N_CORES  # 100000
N_PAD = -(-N_NODES // 128) * 128  # 50048 table rows
W_SRC = 8192  # per-core src window (nodes; window is ~6.3K wide)
NI = 8192  # slots per tile (per dma_gather call)

_cached = {}  # tiles structure -> compiled nc


def _mk_gather(nc, out_ap, in_ap, idxs_ap, num_idxs, elem_size, elem_step, queue_num):
    from concourse import mybir

    eng = nc.gpsimd
    eng._assert_queue_num(queue_num)
    assert in_ap.dtype == out_ap.dtype == mybir.dt.float16
    assert in_ap.ap[0][0] == elem_step
    assert in_ap.ap[-1][1] == elem_size
    _in_ap = eng.lower_ap_dma(in_ap, for_custom_bir_dma=True)
    return eng.add_instruction(
        mybir.InstDMAGatherAnt(
            name=nc.get_next_instruction_name(),
            ins=[
                *_in_ap,
                eng.lower_ap(idxs_ap),
                eng.lower_val_access(eng.to_reg(num_idxs)),
            ],
            outs=[eng.lower_ap(out_ap)],
            transpose=False,
            num_idxs=num_idxs,
            elem_size=elem_size,
            stride_bytes_256=elem_step * 2 // 256,
            gen_mode=0,
            single_packet=False,
            queue_num=queue_num,
            sbuf_tokens_per_rank=0,
            sbuf_free_dim_per_rank=0,
            sbuf_free_dim_pad_per_rank=0,
            sbuf_byte_offset=0,
        )
    )


def _gather128(nc, out_ap, in_ap, idxs_ap, ni, queue_num):
    """128B rows (one fp16 node) from a 256B-stride table view."""
    assert out_ap.ap[0][1] * out_ap.ap[1][1] == ni
    return _mk_gather(nc, out_ap, in_ap, idxs_ap, ni, D, 2 * D, queue_num)


def _gather512(nc, out_ap, in_ap, idxs_ap, ni, queue_num):
    """512B rows (an aligned quad of 4 fp16 nodes); descriptor m fills
    partition m%128, free levels 4*(m//128)..4*(m//128)+3 (= 4 slots)."""
    assert out_ap.ap[0][1] * out_ap.ap[1][1] == ni and ni % 512 == 0
    return _mk_gather(nc, out_ap, in_ap, idxs_ap, ni // 4, 4 * D, 4 * D, queue_num)


def _build(tiles):
    """tiles: ('q', ni) or ('s', s_par, d_par, ni); ni % 128 == 0 (quad
    tiles ni % 512 == 0)."""
    import concourse.bass as bass
    import concourse.tile as tile
    from concourse import bacc, mybir

    T = len(tiles)
    E_DEV = sum(t[-1] for t in tiles)
    nc = bacc.Bacc(
        "TRN2",
        target_bir_lowering=False,
        debug=False,
        num_devices=N_CORES,
        num_swdge_queues=4,
    )
    h_ap = nc.dram_tensor(
        "h", [N_PAD // 2, 2 * D], mybir.dt.float16, kind="ExternalInput"
    ).ap()
    hs_ap = nc.dram_tensor(
        "hsrc", [W_SRC // 2, 2 * D], mybir.dt.float16, kind="ExternalInput"
    ).ap()
    hsp_ap = nc.dram_tensor(
        "hsrcpad", [W_SRC, 2 * D], mybir.dt.float16, kind="ExternalInput"
    ).ap()
    si_ap = nc.dram_tensor(
        "src_idx", [T, 32, NI // 16], mybir.dt.int16, kind="ExternalInput"
    ).ap()
    di_ap = nc.dram_tensor(
        "dst_idx", [T, 32, NI // 16], mybir.dt.int16, kind="ExternalInput"
    ).ap()
    z_ap = nc.dram_tensor("z", [E_DEV, D], mybir.dt.float16, kind="ExternalOutput").ap()

    dtab = {0: h_ap[:, 0:D], 1: h_ap[:, D : 2 * D]}
    stab = {0: hs_ap[:, 0:D], 1: hs_ap[:, D : 2 * D]}
    quad_tab = h_ap.rearrange("(a b) d -> a (b d)", b=2)  # [N_PAD//4, 256]
    squad_tab = hs_ap.rearrange("(a b) d -> a (b d)", b=2)  # [W_SRC//4, 256]
    spad = hsp_ap[:, 0:D]

    with tile.TileContext(nc) as tc:
        with (
            tc.tile_pool(name="ix", bufs=10) as ixp,
            tc.tile_pool(name="ga", bufs=6) as gap,
            tc.tile_pool(name="gb", bufs=6) as gbp,
            tc.tile_pool(name="zt", bufs=6) as ztp,
        ):
            base = 0
            for t, tt in enumerate(tiles):
                ni = tt[-1]
                g = ni // 128
                qs = (t % 2) * 2
                n_si = (ni // 4 if tt[0] == "r" else ni) // 16
                six = ixp.tile([128, n_si], mybir.dt.int16, tag="six")
                nc.sync.dma_start(
                    six[32 * qs : 32 * qs + 32, :], si_ap[t][:, :n_si]
                )
                n_di = (ni // 4 if tt[0] == "q" else ni) // 16
                dix = ixp.tile([128, n_di], mybir.dt.int16, tag="dix")
                nc.sync.dma_start(
                    dix[32 * qs + 32 : 32 * qs + 64, :], di_ap[t][:, :n_di]
                )
                ga = gap.tile([128, g, D], mybir.dt.float16, tag="ga")
                gb = gbp.tile([128, g, D], mybir.dt.float16, tag="gb")
                if tt[0] == "q":
                    _gather128(nc, ga[:], spad, six[:], ni, qs)
                    _gather512(nc, gb[:], quad_tab, dix[:], ni, qs + 1)
                elif tt[0] == "r":
                    _gather512(nc, ga[:], squad_tab, six[:], ni, qs)
                    _gather128(nc, gb[:], dtab[tt[1]], dix[:], ni, qs + 1)
                else:
                    _, s_par, d_par, _ = tt
                    _gather128(nc, ga[:], stab[s_par], six[:], ni, qs)
                    _gather128(nc, gb[:], dtab[d_par], dix[:], ni, qs + 1)
                zt = ztp.tile([128, g, D], mybir.dt.float16, tag="zt")
                nc.vector.tensor_mul(zt[:], ga[:], gb[:])
                z_view = z_ap[base : base + ni, :].rearrange(
                    "(p gd) d -> p (gd d)", p=128
                )
                nc.sync.dma_start(z_view, zt[:])
                base += ni
    nc.compile()
    return nc


def _wrap16(a):
    """[n] int16 gather-sequence -> wrapped [32, n//16] layout: position i
    lives at partition i%16, slot i//16, replicated x2 (RX/TX Q7 cores)."""
    w = a.reshape(-1, 16).T
    return np.ascontiguousarray(np.tile(w, (2, 1)))


def _prepare(src, dst):
    src = np.asarray(src).astype(np.int64)
    dst = np.asarray(dst).astype(np.int64)
    order = np.argsort(src, kind="stable")
    spans = [order[c * E_PER_CORE : (c + 1) * E_PER_CORE] for c in range(N_CORES)]
    n0s = [int(src[sp].min()) & ~3 for sp in spans]

    # --- per-core quad/singles split ---------------------------------
    # qsets[c]: list of (quad, rank) descriptors; qedges[c][m] = 4 edge ids
    # (-1 for absent); sgroups[c][k] = leftover edges, dst-sorted
    qdescs = []
    qslots = []  # per core: [D_c, 4] edge ids (-1 pad)
    sqdescs = []  # per core: [b] -> src-quad idx per desc
    sqslots = []  # per core: [b] -> [Dn, 4] edge ids
    sgroups = []
    for c in range(N_CORES):
        e = spans[c]
        assert src[e].max() - n0s[c] < W_SRC
        d = dst[e]
        o = np.argsort(d, kind="stable")
        e, d = e[o], d[o]
        cnt = np.bincount(d, minlength=N_PAD)
        first = np.zeros(N_PAD, np.int64)
        first[d[np.concatenate([[True], d[1:] != d[:-1]])]] = np.flatnonzero(
            np.concatenate([[True], d[1:] != d[:-1]])
        )
        rank = np.arange(len(d)) - first[d]
        cntm = cnt.reshape(-1, 4)  # [N_PAD//4, 4]
        # set (q, j) taken iff ALL 4 of the quad's nodes have cnt > j: a
        # 512B desc (22.76ns) only beats 4x 128B descs (4x11.38ns) when all
        # 4 slots carry real edges — every empty slot would add a src
        # descriptor and z-store bytes of its own
        k_of = (cntm[d >> 2] > rank[:, None]).sum(1)
        in_q = k_of == 4
        eq, dq, rq = e[in_q], d[in_q], rank[in_q]
        # descriptor id: dense over taken (q, j) pairs, quad-ordered
        setkey = (dq >> 2) * 64 + rq  # rank < 64 always (max multiplicity)
        assert rq.max(initial=0) < 64
        uniq, inv = np.unique(setkey, return_inverse=True)
        Dn = len(uniq)
        slots = np.full((Dn, 4), -1, np.int64)
        slots[inv, dq & 3] = eq
        qdescs.append((uniq >> 6).astype(np.int64))  # quad index per desc
        qslots.append(slots)
        # leftovers, still dst-sorted: mine SRC quads next (full sets over 4
        # consecutive window nodes, all 4 edges sharing dst parity since a
        # tile has one dst table view). Ranking each node's edges in dst
        # order makes same-rank sets cluster by dst percentile, so ordering
        # descs rank-major keeps a coarse dst sweep.
        el = e[~in_q]
        sq_d, sq_s = [], []  # per dst-parity: desc src-quad idx, [Dn,4] edges
        s_left = []
        for b in range(2):
            eb = el[(dst[el] & 1) == b]
            s = src[eb] - n0s[c]
            o2 = np.argsort(s, kind="stable")  # dst order kept within node
            eb, s = eb[o2], s[o2]
            cnt = np.bincount(s, minlength=W_SRC)
            bound = np.concatenate([[True], s[1:] != s[:-1]])
            first = np.zeros(W_SRC, np.int64)
            first[s[bound]] = np.flatnonzero(bound)
            rank = np.arange(len(s)) - first[s]
            cntm = cnt.reshape(-1, 4)
            k_of = (cntm[s >> 2] > rank[:, None]).sum(1)
            in_sq = k_of == 4
            ebq, sq, rq = eb[in_sq], s[in_sq], rank[in_sq]
            assert rq.max(initial=0) < 256
            setkey = rq * (W_SRC // 4) + (sq >> 2)  # rank-major order
            uniq2, inv2 = np.unique(setkey, return_inverse=True)
            slots2 = np.full((len(uniq2), 4), -1, np.int64)
            slots2[inv2, sq & 3] = ebq
            sq_d.append((uniq2 % (W_SRC // 4)).astype(np.int64))
            sq_s.append(slots2)
            s_left.append(eb[~in_sq])
        sqdescs.append(sq_d)
        sqslots.append(sq_s)
        # final singles: 4 parity buckets, re-sorted by dst (the src-quad
        # pass left them src-sorted)
        glist = []
        for kk in range(4):
            ee = s_left[kk & 1]
            ee = ee[(src[ee] & 1) == (kk >> 1)]
            ee = ee[np.argsort(dst[ee], kind="stable")]
            glist.append(ee)
        sgroups.append(glist)

    cap_q = -(-max(len(q) for q in qdescs) // 128) * 128  # descs, %128
    cap_sq = [
        -(-max(len(sqdescs[c][b]) for c in range(N_CORES)) // 128) * 128
        for b in range(2)
    ]
    caps_s = [
        -(-max(len(sgroups[c][k]) for c in range(N_CORES)) // 128) * 128
        for k in range(4)
    ]
    tiles = []
    rem = cap_q * 4  # slots
    while rem > 0:
        ni = min(NI, rem)
        tiles.append(("q", ni))
        rem -= ni
    for b in range(2):
        rem = cap_sq[b] * 4
        while rem > 0:
            ni = min(NI, rem)
            tiles.append(("r", b, ni))
            rem -= ni
    for k in range(4):
        rem = caps_s[k]
        while rem > 0:
            ni = min(NI, rem)
            tiles.append(("s", k >> 1, k & 1, ni))
            rem -= ni
    T = len(tiles)
    E_DEV = sum(t[-1] for t in tiles)
    tile_bases = np.cumsum([0] + [t[-1] for t in tiles])

    in_maps = []
    dev_orig = np.empty((N_CORES, E_DEV), np.int64)
    for c in range(N_CORES):
        # slot -> original edge over the whole device slot space
        orig = np.full(E_DEV, -1, np.int64)
        qd = np.zeros(cap_q, np.int64)  # dst quad idx per desc
        qd[: len(qdescs[c])] = qdescs[c]
        qsl = np.full((cap_q, 4), -1, np.int64)
        qsl[: len(qdescs[c])] = qslots[c]
        sqd, sqsl = [], []
        for b in range(2):
            a = np.zeros(cap_sq[b], np.int64)
            a[: len(sqdescs[c][b])] = sqdescs[c][b]
            sqd.append(a)
            sl = np.full((cap_sq[b], 4), -1, np.int64)
            sl[: len(sqdescs[c][b])] = sqslots[c][b]
            sqsl.append(sl)
        sq_start = [cap_q * 4, cap_q * 4 + cap_sq[0] * 4]
        # singles regions (quad region rows are filled per tile below)
        spos = cap_q * 4 + (cap_sq[0] + cap_sq[1]) * 4
        for k in range(4):
            e = sgroups[c][k]
            orig[spos : spos + len(e)] = e
            spos += caps_s[k]

        si = np.zeros((T, 32, NI // 16), np.int16)
        di = np.zeros((T, 32, NI // 16), np.int16)
        for t, tt in enumerate(tiles):
            ni = tt[-1]
            b = tile_bases[t]
            g = ni // 128
            if tt[0] == "q":
                m0 = b // 4  # first desc of this tile (quad tiles lead)
                descs = qd[m0 : m0 + ni // 4]
                di[t, :, : ni // 64] = _wrap16(descs.astype(np.int16))
                # dst desc m -> partition m%128, z levels 4*(m//128)+r;
                # z row = p*g + level
                mm = np.arange(ni // 4)
                part = mm % 128
                lev = 4 * (mm // 128)
                s_rows = np.full(ni, -1, np.int64)
                edges4 = qsl[m0 : m0 + ni // 4]  # [ni//4, 4]
                for r in range(4):
                    s_rows[part * g + lev + r] = edges4[:, r]
                s16 = np.zeros(ni, np.int64)
                valid = s_rows >= 0
                s16[valid] = src[s_rows[valid]] - n0s[c]
                # src gather position i writes z row (i%128)*g + i//128:
                # emit the src idx sequence in position order
                pos_row = (np.arange(ni) % 128) * g + (np.arange(ni) // 128)
                si[t, :, : ni // 16] = _wrap16(s16[pos_row].astype(np.int16))
                dev_orig[c, b : b + ni] = s_rows
            elif tt[0] == "r":
                bp = tt[1]
                m0 = (b - sq_start[bp]) // 4
                descs = sqd[bp][m0 : m0 + ni // 4]
                si[t, :, : ni // 64] = _wrap16(descs.astype(np.int16))
                # src desc m -> partition m%128, z levels 4*(m//128)+r
                mm = np.arange(ni // 4)
                part = mm % 128
                lev = 4 * (mm // 128)
                s_rows = np.full(ni, -1, np.int64)
                edges4 = sqsl[bp][m0 : m0 + ni // 4]
                for r in range(4):
                    s_rows[part * g + lev + r] = edges4[:, r]
                d16 = np.zeros(ni, np.int64)
                valid = s_rows >= 0
                d16[valid] = dst[s_rows[valid]] >> 1
                pos_row = (np.arange(ni) % 128) * g + (np.arange(ni) // 128)
                di[t, :, : ni // 16] = _wrap16(d16[pos_row].astype(np.int16))
                dev_orig[c, b : b + ni] = s_rows
            else:
                _, s_par, d_par, _ = tt
                oo = orig[b : b + ni]
                s16 = np.where(oo >= 0, (src[np.maximum(oo, 0)] - n0s[c]) >> 1, 0)
                d16 = np.where(oo >= 0, dst[np.maximum(oo, 0)] >> 1, 0)
                si[t, :, : ni // 16] = _wrap16(s16.astype(np.int16))
                di[t, :, : ni // 16] = _wrap16(d16.astype(np.int16))
                tmap = np.arange(ni).reshape(g, 128).T.reshape(-1)
                dev_orig[c, b : b + ni] = oo[tmap]
        in_maps.append({"si": si, "di": di})
    return tiles, in_maps, dev_orig, n0s


def _get_nc(tiles):
    key = tuple(tiles)
    if key not in _cached:
        _cached[key] = _build(list(key))
    return _cached[key]


def _make_in_maps(h, src, dst):
    tiles, idx_maps, dev_orig, n0s = _prepare(src, dst)
    h16 = np.asarray(h, dtype=np.float16)
    hpk = np.zeros((N_PAD // 2, 2 * D), np.float16)
    hpk[: N_NODES // 2] = h16.reshape(N_NODES // 2, 2 * D)
    in_maps = []
    for c, m in enumerate(idx_maps):
        end = min(n0s[c] + W_SRC, N_NODES)
        n = end - n0s[c]
        flat = np.zeros((W_SRC, D), np.float16)
        flat[:n] = h16[n0s[c] : end]
        hs = flat.reshape(W_SRC // 2, 2 * D)
        hsp = np.zeros((W_SRC, 2 * D), np.float16)
        hsp[:, :D] = flat
        in_maps.append(
            {
                "h": hpk,
                "hsrc": hs,
                "hsrcpad": hsp,
                "src_idx": m["si"],
                "dst_idx": m["di"],
            }
        )
    return tiles, in_maps, dev_orig


def kernel(h, src, dst):
    from concourse import bass_utils

    tiles, in_maps, dev_orig = _make_in_maps(h, src, dst)
    nc = _get_nc(tiles)
    res = bass_utils.run_bass_kernel_spmd(nc, in_maps, list(range(N_CORES)))
    out = np.empty((N_EDGES, D), np.float32)
    for c in range(N_CORES):
        zc = res.results[c]["z"]
        valid = dev_orig[c] >= 0
        out[dev_orig[c][valid]] = zc[valid].astype(np.float32)
    return out


# revision 39
# speedup vs baseline: 1.0003x; 1.0003x over previous
"""Edge-parallel GNN u_mul_v kernel for Trainium2 (8 NeuronCores).

z[e, :] = h[src[e], :] * h[dst[e], :]

Sharding: edges are globally sorted by src and cores take contiguous 100K
spans, so each core's src values live in a narrow ~6.3K-node window, shipped
per-core as rebased tables (indices fit the gather's signed int16).

Tables are dense fp16: two nodes per 256B row (h16 as [N_PAD/2, 128]). The
SWDGE gather cost is per descriptor with a 2x penalty under 512B, so a
128B descriptor (one node, elem_size=64 fp16, 256B row stride) costs half a
256B one, and a 512B descriptor (an aligned quad of 4 nodes) costs the same
as two 128B ones while serving up to 4 edge slots. The bass dma_gather
wrapper refuses sub-256B elements (a transpose-path restriction only), so
_gather128/_gather512 build InstDMAGatherAnt directly (the non-transpose
TX/RX descriptors in q7_kernels/extended_inst/dma_gather.cpp carry
arbitrary lengths; only the row stride is 256B-granular).

Per core the edges are split into:
  - quad region: greedy sets over dst quads (nodes 4q..4q+3) with >= 2
    participating nodes; one 512B dst descriptor serves the set's slots
    (descriptor m -> partition m%128, z levels 4*(m//128)+r). Descriptors
    are quad-ordered, so the dst stream stays sequential. The src side of
    these tiles gathers 128B rows from a padded per-core window table.
  - singles region: remaining edges, bucketed by (src parity, dst parity)
    and dst-sorted; both sides gather 128B rows via the even/odd column
    views of the packed tables.
Device programs are SPMD, so region/bucket capacities are padded to the
per-core max.

Per 8192-slot tile: two SWDGE gathers (separate queues), one DVE fp16
multiply, one contiguous HWDGE store of z [E_DEV, 64] fp16 (host converts
to fp32 and applies the inverse edge permutation). Gather index tensors are
shipped once (not 8x): queue q's Q7 pair reads idx only from partitions
[32q, 32q+32).
"""

import numpy as np

N_NODES = 50000
N_EDGES = 800000
D = 64
N_CORES = 8
E_PER_CORE = N_EDGES // N_CORES  # 100000
N_PAD = -(-N_NODES // 128) * 128  # 50048 table rows
W_SRC = 8192  # per-core src window (nodes; window is ~6.3K wide)
NI = 8192  # slots per tile (per dma_gather call)

_cached = {}  # tiles structure -> compiled nc


def _mk_gather(nc, out_ap, in_ap, idxs_ap, num_idxs, elem_size, elem_step, queue_num):
    from concourse import mybir

    eng = nc.gpsimd
    eng._assert_queue_num(queue_num)
    assert in_ap.dtype == out_ap.dtype == mybir.dt.float16
    assert in_ap.ap[0][0] == elem_step
    assert in_ap.ap[-1][1] == elem_size
    _in_ap = eng.lower_ap_dma(in_ap, for_custom_bir_dma=True)
    return eng.add_instruction(
        mybir.InstDMAGatherAnt(
            name=nc.get_next_instruction_name(),
            ins=[
                *_in_ap,
                eng.lower_ap(idxs_ap),
                eng.lower_val_access(eng.to_reg(num_idxs)),
            ],
            outs=[eng.lower_ap(out_ap)],
            transpose=False,
            num_idxs=num_idxs,
            elem_size=elem_size,
            stride_bytes_256=elem_step * 2 // 256,
            gen_mode=0,
            single_packet=False,
            queue_num=queue_num,
            sbuf_tokens_per_rank=0,
            sbuf_free_dim_per_rank=0,
            sbuf_free_dim_pad_per_rank=0,
            sbuf_byte_offset=0,
        )
    )


def _gather128(nc, out_ap, in_ap, idxs_ap, ni, queue_num):
    """128B rows (one fp16 node) from a 256B-stride table view."""
    assert out_ap.ap[0][1] * out_ap.ap[1][1] == ni
    return _mk_gather(nc, out_ap, in_ap, idxs_ap, ni, D, 2 * D, queue_num)


def _gather512(nc, out_ap, in_ap, idxs_ap, ni, queue_num):
    """512B rows (an aligned quad of 4 fp16 nodes); descriptor m fills
    partition m%128, free levels 4*(m//128)..4*(m//128)+3 (= 4 slots)."""
    assert out_ap.ap[0][1] * out_ap.ap[1][1] == ni and ni % 512 == 0
    return _mk_gather(nc, out_ap, in_ap, idxs_ap, ni // 4, 4 * D, 4 * D, queue_num)


def _build(tiles):
    """tiles: ('q', ni) or ('s', s_par, d_par, ni); ni % 128 == 0 (quad
    tiles ni % 512 == 0)."""
    import concourse.bass as bass
    import concourse.tile as tile
    from concourse import bacc, mybir

    T = len(tiles)
    E_DEV = sum(t[-1] for t in tiles)
    nc = bacc.Bacc(
        "TRN2",
        target_bir_lowering=False,
        debug=False,
        num_devices=N_CORES,
        num_swdge_queues=4,
    )
    h_ap = nc.dram_tensor(
        "h", [N_PAD // 2, 2 * D], mybir.dt.float16, kind="ExternalInput"
    ).ap()
    hs_ap = nc.dram_tensor(
        "hsrc", [W_SRC // 2, 2 * D], mybir.dt.float16, kind="ExternalInput"
    ).ap()
    hsp_ap = nc.dram_tensor(
        "hsrcpad", [W_SRC, 2 * D], mybir.dt.float16, kind="ExternalInput"
    ).ap()
    si_ap = nc.dram_tensor(
        "src_idx", [T, 32, NI // 16], mybir.dt.int16, kind="ExternalInput"
    ).ap()
    di_ap = nc.dram_tensor(
        "dst_idx", [T, 32, NI // 16], mybir.dt.int16, kind="ExternalInput"
    ).ap()
    z_ap = nc.dram_tensor("z", [E_DEV, D], mybir.dt.float16, kind="ExternalOutput").ap()

    dtab = {0: h_ap[:, 0:D], 1: h_ap[:, D : 2 * D]}
    stab = {0: hs_ap[:, 0:D], 1: hs_ap[:, D : 2 * D]}
    quad_tab = h_ap.rearrange("(a b) d -> a (b d)", b=2)  # [N_PAD//4, 256]
    squad_tab = hs_ap.rearrange("(a b) d -> a (b d)", b=2)  # [W_SRC//4, 256]
    spad = hsp_ap[:, 0:D]

    with tile.TileContext(nc) as tc:
        with (
            tc.tile_pool(name="ix", bufs=10) as ixp,
            tc.tile_pool(name="ga", bufs=6) as gap,
            tc.tile_pool(name="gb", bufs=6) as gbp,
            tc.tile_pool(name="zt", bufs=6) as ztp,
        ):
            base = 0
            for t, tt in enumerate(tiles):
                ni = tt[-1]
                g = ni // 128
                qs = (t % 2) * 2
                n_si = (ni // 4 if tt[0] == "r" else ni) // 16
                six = ixp.tile([128, n_si], mybir.dt.int16, tag="six")
                nc.sync.dma_start(
                    six[32 * qs : 32 * qs + 32, :], si_ap[t][:, :n_si]
                )
                n_di = (ni // 4 if tt[0] == "q" else ni) // 16
                dix = ixp.tile([128, n_di], mybir.dt.int16, tag="dix")
                nc.sync.dma_start(
                    dix[32 * qs + 32 : 32 * qs + 64, :], di_ap[t][:, :n_di]
                )
                ga = gap.tile([128, g, D], mybir.dt.float16, tag="ga")
                gb = gbp.tile([128, g, D], mybir.dt.float16, tag="gb")
                if tt[0] == "q":
                    _gather128(nc, ga[:], spad, six[:], ni, qs)
                    _gather512(nc, gb[:], quad_tab, dix[:], ni, qs + 1)
                elif tt[0] == "r":
                    _gather512(nc, ga[:], squad_tab, six[:], ni, qs)
                    _gather128(nc, gb[:], dtab[tt[1]], dix[:], ni, qs + 1)
                else:
                    _, s_par, d_par, _ = tt
                    _gather128(nc, ga[:], stab[s_par], six[:], ni, qs)
                    _gather128(nc, gb[:], dtab[d_par], dix[:], ni, qs + 1)
                zt = ztp.tile([128, g, D], mybir.dt.float16, tag="zt")
                z_view = z_ap[base : base + ni, :].rearrange(
                    "(p gd) d -> p (gd d)", p=128
                )
                if t == T - 1 and g >= 2:
                    # drain the final tile in halves so the first store
                    # overlaps the second multiply
                    h1 = g // 2
                    nc.vector.tensor_mul(zt[:, :h1], ga[:, :h1], gb[:, :h1])
                    nc.sync.dma_start(z_view[:, : h1 * D], zt[:, :h1])
                    nc.vector.tensor_mul(zt[:, h1:], ga[:, h1:], gb[:, h1:])
                    nc.sync.dma_start(z_view[:, h1 * D :], zt[:, h1:])
                else:
                    nc.vector.tensor_mul(zt[:], ga[:], gb[:])
                    nc.sync.dma_start(z_view, zt[:])
                base += ni
    nc.compile()
    return nc


def _wrap16(a):
    """[n] int16 gather-sequence -> wrapped [32, n//16] layout: position i
    lives at partition i%16, slot i//16, replicated x2 (RX/TX Q7 cores)."""
    w = a.reshape(-1, 16).T
    return np.ascontiguousarray(np.tile(w, (2, 1)))


def _prepare(src, dst):
    src = np.asarray(src).astype(np.int64)
    dst = np.asarray(dst).astype(np.int64)
    order = np.argsort(src, kind="stable")
    spans = [order[c * E_PER_CORE : (c + 1) * E_PER_CORE] for c in range(N_CORES)]
    n0s = [int(src[sp].min()) & ~3 for sp in spans]

    # --- per-core quad/singles split ---------------------------------
    # qsets[c]: list of (quad, rank) descriptors; qedges[c][m] = 4 edge ids
    # (-1 for absent); sgroups[c][k] = leftover edges, dst-sorted
    qdescs = []
    qslots = []  # per core: [D_c, 4] edge ids (-1 pad)
    sqdescs = []  # per core: [b] -> src-quad idx per desc
    sqslots = []  # per core: [b] -> [Dn, 4] edge ids
    sgroups = []
    for c in range(N_CORES):
        e = spans[c]
        assert src[e].max() - n0s[c] < W_SRC
        d = dst[e]
        o = np.argsort(d, kind="stable")
        e, d = e[o], d[o]
        cnt = np.bincount(d, minlength=N_PAD)
        first = np.zeros(N_PAD, np.int64)
        first[d[np.concatenate([[True], d[1:] != d[:-1]])]] = np.flatnonzero(
            np.concatenate([[True], d[1:] != d[:-1]])
        )
        rank = np.arange(len(d)) - first[d]
        cntm = cnt.reshape(-1, 4)  # [N_PAD//4, 4]
        # set (q, j) taken iff ALL 4 of the quad's nodes have cnt > j: a
        # 512B desc (22.76ns) only beats 4x 128B descs (4x11.38ns) when all
        # 4 slots carry real edges — every empty slot would add a src
        # descriptor and z-store bytes of its own
        k_of = (cntm[d >> 2] > rank[:, None]).sum(1)
        in_q = k_of == 4
        eq, dq, rq = e[in_q], d[in_q], rank[in_q]
        # descriptor id: dense over taken (q, j) pairs, quad-ordered
        setkey = (dq >> 2) * 64 + rq  # rank < 64 always (max multiplicity)
        assert rq.max(initial=0) < 64
        uniq, inv = np.unique(setkey, return_inverse=True)
        Dn = len(uniq)
        slots = np.full((Dn, 4), -1, np.int64)
        slots[inv, dq & 3] = eq
        qdescs.append((uniq >> 6).astype(np.int64))  # quad index per desc
        qslots.append(slots)
        # leftovers, still dst-sorted: mine SRC quads next (full sets over 4
        # consecutive window nodes, all 4 edges sharing dst parity since a
        # tile has one dst table view). Ranking each node's edges in dst
        # order makes same-rank sets cluster by dst percentile, so ordering
        # descs rank-major keeps a coarse dst sweep.
        el = e[~in_q]
        sq_d, sq_s = [], []  # per dst-parity: desc src-quad idx, [Dn,4] edges
        s_left = []
        for b in range(2):
            eb = el[(dst[el] & 1) == b]
            s = src[eb] - n0s[c]
            o2 = np.argsort(s, kind="stable")  # dst order kept within node
            eb, s = eb[o2], s[o2]
            cnt = np.bincount(s, minlength=W_SRC)
            bound = np.concatenate([[True], s[1:] != s[:-1]])
            first = np.zeros(W_SRC, np.int64)
            first[s[bound]] = np.flatnonzero(bound)
            rank = np.arange(len(s)) - first[s]
            cntm = cnt.reshape(-1, 4)
            k_of = (cntm[s >> 2] > rank[:, None]).sum(1)
            in_sq = k_of == 4
            ebq, sq, rq = eb[in_sq], s[in_sq], rank[in_sq]
            assert rq.max(initial=0) < 256
            setkey = rq * (W_SRC // 4) + (sq >> 2)  # rank-major order
            uniq2, inv2 = np.unique(setkey, return_inverse=True)
            slots2 = np.full((len(uniq2), 4), -1, np.int64)
            slots2[inv2, sq & 3] = ebq
            sq_d.append((uniq2 % (W_SRC // 4)).astype(np.int64))
            sq_s.append(slots2)
            s_left.append(eb[~in_sq])
        sqdescs.append(sq_d)
        sqslots.append(sq_s)
        # final singles: 4 parity buckets, re-sorted by dst (the src-quad
        # pass left them src-sorted)
        glist = []
        for kk in range(4):
            ee = s_left[kk & 1]
            ee = ee[(src[ee] & 1) == (kk >> 1)]
            ee = ee[np.argsort(dst[ee], kind="stable")]
            glist.append(ee)
        sgroups.append(glist)

    cap_q = -(-max(len(q) for q in qdescs) // 128) * 128  # descs, %128
    cap_sq = [
        -(-max(len(sqdescs[c][b]) for c in range(N_CORES)) // 128) * 128
        for b in range(2)
    ]
    caps_s = [
        -(-max(len(sgroups[c][k]) for c in range(N_CORES)) // 128) * 128
        for k in range(4)
    ]
    tiles = []
    rem = cap_q * 4  # slots
    while rem > 0:
        ni = min(NI, rem)
        tiles.append(("q", ni))
        rem -= ni
    for b in range(2):
        rem = cap_sq[b] * 4
        while rem > 0:
            ni = min(NI, rem)
            tiles.append(("r", b, ni))
            rem -= ni
    for k in range(4):
        rem = caps_s[k]
        while rem > 0:
            ni = min(NI, rem)
            tiles.append(("s", k >> 1, k & 1, ni))
            rem -= ni
    T = len(tiles)
    E_DEV = sum(t[-1] for t in tiles)
    tile_bases = np.cumsum([0] + [t[-1] for t in tiles])

    in_maps = []
    dev_orig = np.empty((N_CORES, E_DEV), np.int64)
    for c in range(N_CORES):
        # slot -> original edge over the whole device slot space
        orig = np.full(E_DEV, -1, np.int64)
        qd = np.zeros(cap_q, np.int64)  # dst quad idx per desc
        qd[: len(qdescs[c])] = qdescs[c]
        qsl = np.full((cap_q, 4), -1, np.int64)
        qsl[: len(qdescs[c])] = qslots[c]
        sqd, sqsl = [], []
        for b in range(2):
            a = np.zeros(cap_sq[b], np.int64)
            a[: len(sqdescs[c][b])] = sqdescs[c][b]
            sqd.append(a)
            sl = np.full((cap_sq[b], 4), -1, np.int64)
            sl[: len(sqdescs[c][b])] = sqslots[c][b]
            sqsl.append(sl)
        sq_start = [cap_q * 4, cap_q * 4 + cap_sq[0] * 4]
        # singles regions (quad region rows are filled per tile below)
        spos = cap_q * 4 + (cap_sq[0] + cap_sq[1]) * 4
        for k in range(4):
            e = sgroups[c][k]
            orig[spos : spos + len(e)] = e
            spos += caps_s[k]

        si = np.zeros((T, 32, NI // 16), np.int16)
        di = np.zeros((T, 32, NI // 16), np.int16)
        for t, tt in enumerate(tiles):
            ni = tt[-1]
            b = tile_bases[t]
            g = ni // 128
            if tt[0] == "q":
                m0 = b // 4  # first desc of this tile (quad tiles lead)
                descs = qd[m0 : m0 + ni // 4]
                di[t, :, : ni // 64] = _wrap16(descs.astype(np.int16))
                # dst desc m -> partition m%128, z levels 4*(m//128)+r;
                # z row = p*g + level
                mm = np.arange(ni // 4)
                part = mm % 128
                lev = 4 * (mm // 128)
                s_rows = np.full(ni, -1, np.int64)
                edges4 = qsl[m0 : m0 + ni // 4]  # [ni//4, 4]
                for r in range(4):
                    s_rows[part * g + lev + r] = edges4[:, r]
                s16 = np.zeros(ni, np.int64)
                valid = s_rows >= 0
                s16[valid] = src[s_rows[valid]] - n0s[c]
                # src gather position i writes z row (i%128)*g + i//128:
                # emit the src idx sequence in position order
                pos_row = (np.arange(ni) % 128) * g + (np.arange(ni) // 128)
                si[t, :, : ni // 16] = _wrap16(s16[pos_row].astype(np.int16))
                dev_orig[c, b : b + ni] = s_rows
            elif tt[0] == "r":
                bp = tt[1]
                m0 = (b - sq_start[bp]) // 4
                descs = sqd[bp][m0 : m0 + ni // 4]
                si[t, :, : ni // 64] = _wrap16(descs.astype(np.int16))
                # src desc m -> partition m%128, z levels 4*(m//128)+r
                mm = np.arange(ni // 4)
                part = mm % 128
                lev = 4 * (mm // 128)
                s_rows = np.full(ni, -1, np.int64)
                edges4 = sqsl[bp][m0 : m0 + ni // 4]
                for r in range(4):
                    s_rows[part * g + lev + r] = edges4[:, r]
                d16 = np.zeros(ni, np.int64)
                valid = s_rows >= 0
                d16[valid] = dst[s_rows[valid]] >> 1
                pos_row = (np.arange(ni) % 128) * g + (np.arange(ni) // 128)
                di[t, :, : ni // 16] = _wrap16(d16[pos_row].astype(np.int16))
                dev_orig[c, b : b + ni] = s_rows
            else:
                _, s_par, d_par, _ = tt
                oo = orig[b : b + ni]
                s16 = np.where(oo >= 0, (src[np.maximum(oo, 0)] - n0s[c]) >> 1, 0)
                d16 = np.where(oo >= 0, dst[np.maximum(oo, 0)] >> 1, 0)
                si[t, :, : ni // 16] = _wrap16(s16.astype(np.int16))
                di[t, :, : ni // 16] = _wrap16(d16.astype(np.int16))
                tmap = np.arange(ni).reshape(g, 128).T.reshape(-1)
                dev_orig[c, b : b + ni] = oo[tmap]
        in_maps.append({"si": si, "di": di})
    return tiles, in_maps, dev_orig, n0s


def _get_nc(tiles):
    key = tuple(tiles)
    if key not in _cached:
        _cached[key] = _build(list(key))
    return _cached[key]


def _make_in_maps(h, src, dst):
    tiles, idx_maps, dev_orig, n0s = _prepare(src, dst)
    h16 = np.asarray(h, dtype=np.float16)
    hpk = np.zeros((N_PAD // 2, 2 * D), np.float16)
    hpk[: N_NODES // 2] = h16.reshape(N_NODES // 2, 2 * D)
    in_maps = []
    for c, m in enumerate(idx_maps):
        end = min(n0s[c] + W_SRC, N_NODES)
        n = end - n0s[c]
        flat = np.zeros((W_SRC, D), np.float16)
        flat[:n] = h16[n0s[c] : end]
        hs = flat.reshape(W_SRC // 2, 2 * D)
        hsp = np.zeros((W_SRC, 2 * D), np.float16)
        hsp[:, :D] = flat
        in_maps.append(
            {
                "h": hpk,
                "hsrc": hs,
                "hsrcpad": hsp,
                "src_idx": m["si"],
                "dst_idx": m["di"],
            }
        )
    return tiles, in_maps, dev_orig


def kernel(h, src, dst):
    from concourse import bass_utils

    tiles, in_maps, dev_orig = _make_in_maps(h, src, dst)
    nc = _get_nc(tiles)
    res = bass_utils.run_bass_kernel_spmd(nc, in_maps, list(range(N_CORES)))
    out = np.empty((N_EDGES, D), np.float32)
    for c in range(N_CORES):
        zc = res.results[c]["z"]
        valid = dev_orig[c] >= 0
        out[dev_orig[c][valid]] = zc[valid].astype(np.float32)
    return out
